# revision 46
# baseline (speedup 1.0000x reference)
"""Trainium2 Bass kernel for BiLSTM-CRF log-likelihood.

Pipeline (per core, pure data-parallel over batch: 8 of 64 sequences/core):
  concat(hid_a,hid_b) -> LN -> 4x conv1d(k=1..4)+relu -> LN -> BiLSTM(256)
  -> dense(20) -> CRF log-likelihood  -> [B] scores.

Device layout is feature-major ("transposed"): features on SBUF partitions,
(batch, time) on the free axis.  All big matmuls run in bf16 with fp32 PSUM
accumulation.  The CRF forward recursion runs in exp-space:
    a_t = (a_{t-1} @ exp(trans)) * exp(emit_t + bd - sigma)
with a constant per-step rescale exp(-sigma) keeping a ~ O(1), so
log Z = log(sum a_{T-1}) + T*sigma.  LSTM recurrence keeps Wh stationary
(8 m-tiles x 2 k-tiles of 128), batch in the moving free dim.

The attention mask is all-ones and no token id is 0 under the problem's input
distribution (randint low=1, mask fill=ones); the device kernel assumes that
and a host-side numpy fallback handles any other input.
"""

import os
import sys
from contextlib import ExitStack

import numpy as np

for _p in ("/opt/trn_rl_repo", "/root/.axon_site/_ro/trn_rl_repo"):
    if os.path.isdir(_p) and _p not in sys.path:
        sys.path.append(_p)

import ml_dtypes  # noqa: E402

import concourse.bass as bass  # noqa: E402
import concourse.tile as tile  # noqa: E402
from concourse import bacc, mybir  # noqa: E402
from concourse._compat import with_exitstack  # noqa: E402
from concourse.alu_op_type import AluOpType  # noqa: E402
from concourse.bass import ds, ts  # noqa: E402

F32 = mybir.dt.float32
BF16 = mybir.dt.bfloat16
FP8 = mybir.dt.float8e4
DR = mybir.MatmulPerfMode.DoubleRow
FP8NP = ml_dtypes.float8_e4m3
AF = mybir.ActivationFunctionType
OP = AluOpType
BFNP = ml_dtypes.bfloat16

# problem dims
B, T_FULL, D_BERT, LBL, H = 64, 512, 768, 20, 256
D = 2 * D_BERT            # 1536, LN1/conv input features
C = 192
C4 = 4 * C                # 768, conv concat channels
G4 = 4 * H                # 1024, lstm gate width
NCORE = 8
BL = B // NCORE           # 8 sequences per core
KD = D // 128             # 12
KC = C4 // 128            # 6
MG = G4 // 128            # 8
KH = H // 128             # 2
KW = (2 * H) // 128       # 4 (dense k-tiles)
SIGMA = 3.0
EPS = 1e-5

# conv taps, grouped by time offset.  TF/XLA SAME padding:
# K=1 -> {0}; K=2 -> {0,+1}; K=3 -> {-1,0,+1}; K=4 -> {-1,0,+1,+2}
# concat channel blocks: conv1 0:192, conv2 192:384, conv3 384:576, conv4 576:768
# 128-wide m-blocks and which offsets are active in each:
ACTIVE = {0: [0], 1: [0, 1], 2: [0, 1], 3: [-1, 0, 1], 4: [-1, 0, 1, 2], 5: [-1, 0, 1, 2]}
PAIRS = [(mb, off) for mb in range(6) for off in ACTIVE[mb]]  # 16 (mb,off) pairs
NPAIR = len(PAIRS)
# gate reorder: keras order i,f,g,o -> device order i,f,o,g with the g block
# last and pre-scaled by 2: one ACT Sigmoid covers the whole z tile and the
# DVE recovers tanh(g) = 2*sigmoid(2g) - 1 with a cheap bf16 tensor_scalar
PERM = np.r_[0:H, H:2 * H, 3 * H:4 * H, 2 * H:3 * H]
FP8_CONV = True            # conv1d matmuls in fp8e4m3 DoubleRow (k=256/instr)
XSC = 8.0                  # LN1 output pre-scale (fp8 dynamic range)
WSC = 64.0                 # conv weight pre-scale (fp8 dynamic range)
FP8_XW = True              # xW matmuls in fp8e4m3 DoubleRow
XSC2 = 8.0                 # LN2 output pre-scale
WSC2 = 64.0                # wx weight pre-scale
FP8_H = False              # fp8 DoubleRow Wh measured WORSE (FD=8 kills FWL: LDW 213ns
                           # vs 26ns dominates the on-chain burst; +436us) — keep bf16
TGACT = False              # tanh(g) via a 2nd ACT measured WORSE (+220us): 6 ACTs/step
                           # saturate the Scalar queue and lengthen the chain — keep the
                           # DVE 2s-1 conversion (4 ACTs/step)
NHEAT = 3                  # HAM-heater matmuls per LSTM step on rotating PSUM tiles
                           # (a single tile WAW-serialized them at ~270ns: +246us)
WDSC = 64.0                # wd weight pre-scale (fp8), compensated via oht/bd host packing
BLP = 16                   # hbuf batch-axis padding so the k-pair stride is 16B


# ---------------------------------------------------------------- device build

@with_exitstack
def _emit(ctx, tc, io, T, TCH):
    """Emit the full program. io: dict name -> dram AP."""
    nc = tc.nc
    TP = T + 3  # padded time axis (1 left, 2 right) for conv taps
    TP8 = T + 16  # fp8 pair-tile time axis: 16B-aligned stride for DoubleRow APs

    per = ctx.enter_context(tc.tile_pool(name="persist", bufs=1))

    # --- persistent constants / weights -> SBUF
    ones1b = per.tile([128, 1], BF16)
    nc.any.memset(ones1b[:], 1.0)
    ones1f = per.tile([128, 1], F32)
    nc.any.memset(ones1f[:], 1.0)
    onesrowf = per.tile([1, 128], F32)
    nc.any.memset(onesrowf[:], 1.0)
    ones20 = per.tile([20, 1], F32)
    nc.any.memset(ones20[:], 1.0)
    epscol = per.tile([1, 1], F32)
    nc.any.memset(epscol[:], EPS)

    if FP8_H:
        wh_sb = per.tile([128, KH, 2 * G4], FP8)
        for k in range(KH):
            nc.sync.dma_start(wh_sb[:, k], io["wh"][ds(k * 128, 128), :])
        wd_sb = per.tile([128, KW, LBL], FP8)
        for k in range(KW):
            nc.sync.dma_start(wd_sb[:, k], io["wd"][ds(k * 128, 128), :])
    else:
        wh_sb = per.tile([128, KH, 2 * G4], BF16)
        nc.sync.dma_start(wh_sb[:], io["wh"].rearrange("(ko p) m -> p ko m", p=128))
        wd_sb = per.tile([128, KW, LBL], BF16)
        nc.sync.dma_start(wd_sb[:], io["wd"].rearrange("(ko p) m -> p ko m", p=128))
    bz_sb = per.tile([128, 2, MG], F32)
    nc.sync.dma_start(bz_sb[:], io["bz"])
    bd_sb = per.tile([20, 1], F32)
    nc.sync.dma_start(bd_sb[:], io["bd"])
    bdm_sb = per.tile([20, 1], F32)
    nc.sync.dma_start(bdm_sb[:], io["bdm"])
    trans_sb = per.tile([20, 20], F32)
    nc.sync.dma_start(trans_sb[:], io["trans"])
    transT_sb = per.tile([20, 20], F32)
    nc.sync.dma_start(transT_sb[:], io["transT"])

    # persistent state across phases (time-major so the per-step h write is a
    # contiguous [128, KH*BL] slice -> DVE 2x mode).  With FP8_H the batch
    # axis pads to 16 so the k-pair axis stride is 16B (DoubleRow AP rule).
    HDT = FP8 if FP8_H else BF16
    HBL = BLP if FP8_H else BL
    hbuf_f = per.tile([128, T + 1, KH, HBL], HDT)
    hbuf_b = per.tile([128, T + 1, KH, HBL], HDT)
    esb = per.tile([20, BL, T], BF16)      # exp(emit + bd - sigma)
    unacc = per.tile([20, BL], F32)
    binacc = per.tile([20, BL], F32)

    dram = ctx.enter_context(tc.tile_pool(name="dram", bufs=1, space="DRAM"))
    xwt = dram.tile([2, G4, BL, T], BF16)  # x@Wx + bias, gate-major, per dir

    # =================================================== phase A: LN1/conv/LN2/xW
    with ExitStack() as pa:
        wpool = pa.enter_context(tc.tile_pool(name="wconv", bufs=1))
        if FP8_CONV:
            convw = wpool.tile([128, KD // 2, 2, NPAIR * 128], FP8)
            for k in range(KD):
                nc.sync.dma_start(convw[:, k // 2, k % 2],
                                  io["convp"][ds(k * 128, 128), :])
        else:
            convw = wpool.tile([128, KD, NPAIR * 128], BF16)
            for k in range(KD):
                nc.sync.dma_start(convw[:, k], io["convp"][ds(k * 128, 128), :])
        if FP8_XW:
            wx_sb = wpool.tile([128, KC // 2, 2, 2 * G4], FP8)
            for k in range(KC):
                nc.sync.dma_start(wx_sb[:, k // 2, k % 2],
                                  io["wx"][ds(k * 128, 128), :])
        else:
            wx_sb = wpool.tile([128, KC, 2 * G4], BF16)
            nc.sync.dma_start(wx_sb[:], io["wx"].rearrange("(ko p) m -> p ko m", p=128))
        g1_sb = wpool.tile([128, KD], F32)
        nc.sync.dma_start(g1_sb[:], io["g1"])
        g2_sb = wpool.tile([128, KC], F32)
        nc.sync.dma_start(g2_sb[:], io["g2"])
        bcv_sb = wpool.tile([128, 6], F32)
        nc.sync.dma_start(bcv_sb[:], io["bconv"])

        sqp = pa.enter_context(tc.tile_pool(name="sq", bufs=6))
        tmpp = pa.enter_context(tc.tile_pool(name="lntmp", bufs=6))
        xpp = pa.enter_context(tc.tile_pool(name="xp", bufs=2 * KD))
        xp8p = pa.enter_context(tc.tile_pool(name="xp8", bufs=KD))
        cvp = pa.enter_context(tc.tile_pool(name="cvr", bufs=12))
        cv8p = pa.enter_context(tc.tile_pool(name="cv8", bufs=6))
        stgp = pa.enter_context(tc.tile_pool(name="stage", bufs=3))
        smallp = pa.enter_context(tc.tile_pool(name="lnsmall", bufs=2))
        sumps = pa.enter_context(tc.tile_pool(name="sums", bufs=1, space="PSUM"))
        bcps = pa.enter_context(tc.tile_pool(name="bcast", bufs=1, space="PSUM"))
        cvps = pa.enter_context(tc.tile_pool(name="cvps", bufs=2, space="PSUM"))
        xwps = pa.enter_context(tc.tile_pool(name="xwps", bufs=2, space="PSUM"))

        def layer_norm_T(xin, nk, gg, out_of):
            """Feature-major layernorm over nk*128 features, in-place capable.
            xin: list of [128, T] APs; out_of(k) -> output AP (may alias xin[k])."""
            s1 = sumps.tile([1, T], F32, tag="s1")
            s2 = sumps.tile([1, T], F32, tag="s2")
            for k in range(nk):
                nc.tensor.matmul(s1[:], ones1b[:], xin[k], start=(k == 0), stop=(k == nk - 1))
            for k in range(nk):
                sq = sqp.tile([128, T], BF16, tag="sq")
                # x*x on DVE (bf16 2x) instead of ACT Square: ~330ns vs ~720ns,
                # and keeps the Scalar engine free for the relu/LN broadcasts
                nc.vector.tensor_tensor(sq[:], xin[k], xin[k], OP.mult)
                nc.tensor.matmul(s2[:], ones1b[:], sq[:], start=(k == 0), stop=(k == nk - 1))
            nf = float(nk * 128)
            mu = smallp.tile([1, T], F32, tag="mu")
            nc.scalar.mul(mu[:], s1[:], 1.0 / nf)
            mu2 = smallp.tile([1, T], F32, tag="mu2")
            nc.vector.tensor_tensor(mu2[:], mu[:], mu[:], OP.mult)
            varr = smallp.tile([1, T], F32, tag="varr")
            nc.vector.scalar_tensor_tensor(varr[:], s2[:], 1.0 / nf, mu2[:], OP.mult, OP.subtract)
            # rsqrt via exp(-0.5*ln(v+eps)): Ln/Exp share one ACT table with
            # Square/Relu/Identity, and this avoids the slow DVE reciprocal
            lnv = smallp.tile([1, T], F32, tag="lnv")
            nc.scalar.activation(lnv[:], varr[:], AF.Ln, bias=epscol[0:1, 0:1])
            rr = smallp.tile([1, T], F32, tag="rr")
            nc.scalar.activation(rr[:], lnv[:], AF.Exp, scale=-0.5)
            mub_ps = bcps.tile([128, T], F32, tag="mub")
            nc.tensor.matmul(mub_ps[:], onesrowf[:], mu[:], start=True, stop=True)
            rb_ps = bcps.tile([128, T], F32, tag="rb")
            nc.tensor.matmul(rb_ps[:], onesrowf[:], rr[:], start=True, stop=True)
            # bf16 SBUF copies of the broadcasts: the per-k DVE ops below then
            # run in 2x mode (all-SBUF, all-16-bit operands)
            mub = tmpp.tile([128, T], BF16, tag="mubs")
            nc.scalar.activation(mub[:], mub_ps[:], AF.Identity)
            rb = tmpp.tile([128, T], BF16, tag="rbs")
            nc.scalar.activation(rb[:], rb_ps[:], AF.Identity)
            for k in range(nk):
                t1 = tmpp.tile([128, T], BF16, tag="lnt")
                nc.vector.tensor_tensor(t1[:], xin[k], mub[:], OP.subtract)
                nc.vector.tensor_tensor(t1[:], t1[:], rb[:], OP.mult)
                nc.vector.tensor_scalar_mul(out_of(k), t1[:], gg[:, k:k + 1])

        def stage_load(b):
            xp = []
            for k in range(KD):
                t = xpp.tile([128, TP], BF16, tag="xp")
                nc.sync.dma_start(t[:, 1:T + 1], io["hidT"][ts(k, 128), b, :])
                xp.append(t)
            return xp

        def stage_ln1(xp):
            if FP8_CONV:
                xp8 = []
                for j in range(KD // 2):
                    t8 = xp8p.tile([128, 2, TP8], FP8, tag="xp8")
                    nc.any.memset(t8[:, :, 0:1], 0.0)
                    nc.any.memset(t8[:, :, T + 1:TP8], 0.0)
                    xp8.append(t8)
                layer_norm_T([xp[k][:, 1:T + 1] for k in range(KD)],
                             KD, g1_sb,
                             lambda k: xp8[k // 2][:, k % 2, 1:T + 1])
                return xp8
            for k in range(KD):
                nc.any.memset(xp[k][:, 0:1], 0.0)
                nc.any.memset(xp[k][:, T + 1:TP], 0.0)
            layer_norm_T([xp[k][:, 1:T + 1] for k in range(KD)],
                         KD, g1_sb, lambda k: xp[k][:, 1:T + 1])
            return xp

        def stage_conv(xp_or_xp8):
            xp8 = xp = xp_or_xp8
            cvr = []
            for mb in range(6):
                cv = cvps.tile([128, T], F32, tag="cv")
                mms = [(p, off) for p, (mb2, off) in enumerate(PAIRS) if mb2 == mb]
                if FP8_CONV:
                    n_mm = len(mms) * (KD // 2)
                    i = 0
                    for p, off in mms:
                        for kj in range(KD // 2):
                            nc.tensor.matmul(
                                cv[:], convw[:, kj, :, ds(p * 128, 128)],
                                xp8[kj][:, :, 1 + off: 1 + off + T],
                                start=(i == 0), stop=(i == n_mm - 1),
                                perf_mode=DR)
                            i += 1
                    relu_scale = 1.0 / (XSC * WSC)
                else:
                    n_mm = len(mms) * KD
                    i = 0
                    for p, off in mms:
                        for k in range(KD):
                            nc.tensor.matmul(
                                cv[:], convw[:, k, ds(p * 128, 128)],
                                xp[k][:, 1 + off: 1 + off + T],
                                start=(i == 0), stop=(i == n_mm - 1))
                            i += 1
                    relu_scale = 1.0
                out = cvp.tile([128, T], BF16, tag="cvr")
                nc.scalar.activation(out[:], cv[:], AF.Relu,
                                     bias=bcv_sb[:, mb:mb + 1], scale=relu_scale)
                cvr.append(out)
            return cvr

        def stage_ln2(cvr):
            if FP8_XW:
                cv8 = []
                for j in range(KC // 2):
                    t8 = cv8p.tile([128, 2, T], FP8, tag="cv8")
                    cv8.append(t8)
                layer_norm_T([cvr[k][:] for k in range(KC)],
                             KC, g2_sb, lambda k: cv8[k // 2][:, k % 2, :])
                return cv8
            layer_norm_T([cvr[k][:] for k in range(KC)],
                         KC, g2_sb, lambda k: cvr[k][:])
            return cvr

        def stage_xw(b, cvx):
            for d in range(2):
                for m in range(MG):
                    xw = xwps.tile([128, T], F32, tag="xw")
                    if FP8_XW:
                        for kj in range(KC // 2):
                            nc.tensor.matmul(
                                xw[:], wx_sb[:, kj, :, ds(d * G4 + m * 128, 128)],
                                cvx[kj][:],
                                start=(kj == 0), stop=(kj == KC // 2 - 1),
                                perf_mode=DR)
                        xw_scale = 1.0 / (XSC2 * WSC2)
                    else:
                        for k in range(KC):
                            nc.tensor.matmul(
                                xw[:], wx_sb[:, k, ds(d * G4 + m * 128, 128)], cvx[k][:],
                                start=(k == 0), stop=(k == KC - 1))
                        xw_scale = 1.0
                    stg = stgp.tile([128, T], BF16, tag="stg")
                    # de-scale + bias-add + downcast in one ACT (Scalar is idle here)
                    nc.scalar.activation(stg[:], xw[:], AF.Identity,
                                         bias=bz_sb[:, d, m:m + 1], scale=xw_scale)
                    nc.sync.dma_start(xwt[d, ds(m * 128, 128), b, :], stg[:])

        # sequential per-sequence emission (a cross-sequence software pipeline
        # was measured WORSE: +380us)
        for b in range(BL):
            xp = stage_load(b)
            xp8 = stage_ln1(xp)
            cvr = stage_conv(xp8)
            cvx = stage_ln2(cvr)
            stage_xw(b, cvx)

    # =================================================== phase B: BiLSTM
    with ExitStack() as pb:
        xchp = pb.enter_context(tc.tile_pool(name="xch", bufs=2))
        zsp = pb.enter_context(tc.tile_pool(name="zs", bufs=2))
        gsp = pb.enter_context(tc.tile_pool(name="gs", bufs=4))
        ctp = pb.enter_context(tc.tile_pool(name="ct", bufs=4))
        ttp = pb.enter_context(tc.tile_pool(name="tt", bufs=6))
        zps = pb.enter_context(tc.tile_pool(name="zps", bufs=2, space="PSUM"))
        htps = pb.enter_context(tc.tile_pool(name="heat", bufs=1, space="PSUM"))

        nc.any.memset(hbuf_f[:, 0], 0.0)
        nc.any.memset(hbuf_b[:, T], 0.0)
        c_cur = [None, None]
        for d in range(2):
            cz = ctp.tile([128, KH, BL], BF16, tag=f"c{d}")
            nc.any.memset(cz[:], 0.0)
            c_cur[d] = cz

        def dma_chunk(t, d, sl):
            for m in range(MG):
                nc.sync.dma_start(t[:, m], xwt[d, ds(m * 128, 128), :, sl])

        nch = T // TCH
        xch_cur = [None, None]
        for d in range(2):
            t = xchp.tile([128, MG, BL, TCH], BF16, tag=f"xch{d}")
            dma_chunk(t, d, ds(0, TCH) if d == 0 else ds(T - TCH, TCH))
            xch_cur[d] = t

        # identity to fold the precomputed xW step-slice into PSUM on the PE,
        # so ACT reads z straight from PSUM (no VE add on the critical chain)
        from concourse.masks import make_identity
        ident = ctp.tile([128, 128], BF16, tag="ident")
        make_identity(nc, ident[:])

        hb = [hbuf_f, hbuf_b]
        # gate column order is i,f,o,g (PERM): the xW step-slice folds into the
        # z PSUM tile with ONE identity matmul (free=64) per dir, then per-m
        # Wh matmuls accumulate; one ACT Sigmoid covers i,f,o and one ACT Tanh
        # covers g.  The two directions' chains interleave so each hides in
        # the other's latency gaps.
        heat_ps = []
        if NHEAT:
            # one PSUM tile per heater slot: no WAW chain between consecutive
            # heater MMs (a single tile serialized them at ~270ns each)
            for j in range(NHEAT):
                heat_t = htps.tile([128, 256], F32, tag=f"heat{j}")
                heat_ps.append(heat_t)
        for step in range(T):
            if NHEAT and step > 0:
                # HAM heater: keep the PE clock-gate warm through the chain
                # gap (these run while the PE waits on the h-writes)
                for j in range(NHEAT):
                    nc.tensor.matmul(heat_ps[j][:], ident[:], wh_sb[:, 0, 0:256],
                                     start=True, stop=True)
            zp2, gs2, tg2 = [None, None], [None, None], [None, None]
            for d in range(2):
                tt = step if d == 0 else T - 1 - step
                ci = step // TCH
                tl = step % TCH
                if tl == 0 and ci + 1 < nch:
                    nxt = xchp.tile([128, MG, BL, TCH], BF16, tag=f"xch{d}")
                    dma_chunk(nxt, d,
                              ds((ci + 1) * TCH, TCH) if d == 0 else ds(T - (ci + 2) * TCH, TCH))
                    xch_cur[d] = (xch_cur[d], nxt)
                xc = xch_cur[d][0] if isinstance(xch_cur[d], tuple) else xch_cur[d]
                tcl = tl if d == 0 else TCH - 1 - tl
                hcol = tt if d == 0 else tt + 1
                zp = zps.tile([128, MG * BL], F32, tag=f"zp{d}")
                nc.tensor.matmul(zp[:], ident[:], xc[:, :, :, tcl],
                                 start=True, stop=False)
                if FP8_H:
                    # DoubleRow folds both k-tiles into one MM: 8 on-chain
                    # matmuls per dir-step instead of 16
                    for m in range(MG):
                        nc.tensor.matmul(
                            zp[:, ds(m * BL, BL)],
                            wh_sb[:, :, ds(d * G4 + m * 128, 128)],
                            hb[d][:, hcol, :, 0:BL],
                            start=False, stop=(m == MG - 1), perf_mode=DR)
                else:
                    for m in range(MG):
                        for k in range(KH):
                            nc.tensor.matmul(
                                zp[:, ds(m * BL, BL)],
                                wh_sb[:, k, ds(d * G4 + m * 128, 128)],
                                hb[d][:, hcol, k, :],
                                start=False,
                                stop=(m == MG - 1 and k == KH - 1))
                zp2[d] = zp
            for d in range(2):
                if TGACT:
                    # tanh(g) straight off the z PSUM g-slice (g pre-scaled x2
                    # in the packing, so scale=0.5 recovers tanh(g)); emitted
                    # before the sigmoid so both land by the time t1 needs them
                    tgt = gsp.tile([128, KH, BL], BF16, tag=f"tga{d}")
                    nc.scalar.activation(
                        tgt[:],
                        zp2[d][:, ds(6 * BL, KH * BL)].rearrange("p (m b) -> p m b", m=KH),
                        AF.Tanh, scale=0.5)
                    tg2[d] = tgt
                    gs = gsp.tile([128, 6, BL], BF16, tag=f"gs{d}")
                    nc.scalar.activation(gs[:],
                                         zp2[d][:, 0:6 * BL].rearrange("p (m b) -> p m b", m=6),
                                         AF.Sigmoid)
                else:
                    gs = gsp.tile([128, MG, BL], BF16, tag=f"gs{d}")
                    nc.scalar.activation(gs[:],
                                         zp2[d][:].rearrange("p (m b) -> p m b", m=MG),
                                         AF.Sigmoid)
                gs2[d] = gs
            cn2 = [None, None]
            for d in range(2):
                gs = gs2[d]  # i=0:2, f=2:4, o=4:6 (+ g=6:8 as sigmoid(2g) if not TGACT)
                if TGACT:
                    tg = tg2[d]
                else:
                    tg = ttp.tile([128, KH, BL], BF16, tag=f"tg{d}")
                    nc.vector.tensor_scalar(tg[:], gs[:, 6:8, :], 2.0, -1.0, OP.mult, OP.add)
                t2 = ttp.tile([128, KH, BL], BF16, tag=f"t2{d}")
                nc.vector.scalar_tensor_tensor(t2[:], c_cur[d][:], 0.0, gs[:, 2:4, :], OP.bypass, OP.mult)
                t1 = ttp.tile([128, KH, BL], BF16, tag=f"t1{d}")
                nc.vector.scalar_tensor_tensor(t1[:], gs[:, 0:2, :], 0.0, tg[:], OP.bypass, OP.mult)
                cn = ctp.tile([128, KH, BL], BF16, tag=f"c{d}")
                nc.vector.tensor_tensor(cn[:], t1[:], t2[:], OP.add)
                c_cur[d] = cn
                thc = ttp.tile([128, KH, BL], BF16, tag=f"thc{d}")
                nc.scalar.activation(thc[:], cn[:], AF.Tanh)
                cn2[d] = thc
            for d in range(2):
                tt = step if d == 0 else T - 1 - step
                wcol = tt + 1 if d == 0 else tt
                nc.vector.scalar_tensor_tensor(
                    hb[d][:, wcol, :, 0:BL], gs2[d][:, 4:6, :], 0.0, cn2[d][:],
                    OP.bypass, OP.mult)
                tl = step % TCH
                if tl == TCH - 1 and isinstance(xch_cur[d], tuple):
                    xch_cur[d] = xch_cur[d][1]

    # =================================================== phase C: logits + CRF
    with ExitStack() as pc:
        ohtp = pc.enter_context(tc.tile_pool(name="oht", bufs=2))
        ohkp = pc.enter_context(tc.tile_pool(name="ohk", bufs=8))
        dmp = pc.enter_context(tc.tile_pool(name="dump", bufs=2))
        crfp = pc.enter_context(tc.tile_pool(name="crf", bufs=4))
        emps = pc.enter_context(tc.tile_pool(name="emps", bufs=2, space="PSUM"))
        cbps = pc.enter_context(tc.tile_pool(name="cbps", bufs=1, space="PSUM"))
        apps = pc.enter_context(tc.tile_pool(name="apps", bufs=3, space="PSUM"))
        fips = pc.enter_context(tc.tile_pool(name="fips", bufs=1, space="PSUM"))

        # k-tiles over time for the bigram matmuls (partial tile for small T)
        kt_sizes = [128] * (T // 128) + ([T % 128] if T % 128 else [])
        # prefetch all one-hot tensors up front so the bigram work (emitted
        # interleaved into the scan below) never waits on DMA
        oht_all, ohp_all, ohn_all = [], [], []
        for b in range(BL):
            oht = ohtp.tile([20, T], F32, tag=f"oht{b}")
            nc.sync.dma_start(oht[:], io["ohT"][:, b, :])
            oht_all.append(oht)
            ohp_t = ohkp.tile([128, len(kt_sizes), 20], BF16, tag=f"ohp{b}")
            ohn_t = ohkp.tile([128, len(kt_sizes), 20], BF16, tag=f"ohn{b}")
            for k, ksz in enumerate(kt_sizes):
                nc.sync.dma_start(ohp_t[:ksz, k], io["ohp"][b, ds(k * 128, ksz), :])
                nc.sync.dma_start(ohn_t[:ksz, k], io["ohn"][b, ds(k * 128, ksz), :])
            ohp_all.append(ohp_t)
            ohn_all.append(ohn_t)
        for b in range(BL):
            em = emps.tile([20, T], F32, tag="em")
            for k in range(KW):
                rhs = (hbuf_f[:, 1:T + 1, k, b] if k < KH
                       else hbuf_b[:, 0:T, k - KH, b])
                nc.tensor.matmul(em[:], wd_sb[:, k, :], rhs, start=(k == 0), stop=(k == KW - 1))
            nc.scalar.activation(esb[:, b, :], em[:], AF.Exp, bias=bdm_sb[:, 0:1],
                                 scale=(1.0 / WDSC if FP8_H else 1.0))
            dump = dmp.tile([20, T], F32, tag="dump")
            nc.vector.scalar_tensor_tensor(
                dump[:], em[:], bd_sb[:, 0:1], oht_all[b][:], OP.add, OP.mult,
                accum_out=unacc[:, b:b + 1])
            cb = cbps.tile([20, 20], F32, tag="cb")
            for k, ksz in enumerate(kt_sizes):
                nc.tensor.matmul(cb[:], ohp_all[b][:ksz, k], ohn_all[b][:ksz, k],
                                 start=(k == 0), stop=(k == len(kt_sizes) - 1))
            dump2 = dmp.tile([20, 20], F32, tag="dump2")
            nc.vector.scalar_tensor_tensor(
                dump2[:], cb[:], 0.0, trans_sb[:], OP.bypass, OP.mult,
                accum_out=binacc[:, b:b + 1])

        # forward alpha and backward beta exp-space scans meet in the middle:
        # alpha_t = (E^T a_{t-1}) * e_t climbs t=1..TM-1, beta_t = E (e_{t+1} *
        # beta_{t+1}) descends t=T-2..TM-1, then Z = sum_j alpha[j]*beta[j].
        # 4 independent PE->VE->PE chains (2 batch halves x alpha/beta) hide
        # each other's latency; e-factor count stays T so the -T*SIGMA
        # correction is unchanged.
        TM = T // 2
        E_sb = crfp.tile([20, 20], F32, tag="E")
        nc.scalar.activation(E_sb[:], trans_sb[:], AF.Exp)
        E2_sb = crfp.tile([20, 20], F32, tag="E2")
        nc.scalar.activation(E2_sb[:], transT_sb[:], AF.Exp)
        # full batch per chain: the alpha and beta scans are already two
        # independent PE->VE->PE chains that hide each other's latency, and
        # the scan is DVE-throughput-bound, so fewer/wider DVE ops win
        a_cur = crfp.tile([20, BL], F32, tag="a0")
        nc.vector.tensor_copy(a_cur[:], esb[:, :, 0])
        b_cur = crfp.tile([20, BL], F32, tag="u0")
        nc.vector.tensor_copy(b_cur[:], esb[:, :, T - 1])
        for s in range(1, TM + 1):
            ps = apps.tile([20, 2, BL], F32, tag="scanps")
            if s <= TM - 1:
                # tile_position pins the 20x20 MM to one 32x32 subarray so the
                # systolic drain (on the scan's critical cycle) is ~32 rows
                nc.tensor.matmul(ps[:, 0], E_sb[:], a_cur[:], start=True, stop=True,
                                 tile_position=(0, 0))
                a_nxt = crfp.tile([20, BL], F32, tag="a")
                nc.vector.scalar_tensor_tensor(
                    a_nxt[:], ps[:, 0], 0.0, esb[:, :, s], OP.bypass, OP.mult)
                a_cur = a_nxt
            tb = T - 1 - s  # beta index produced this slot: 510 .. 255
            nc.tensor.matmul(ps[:, 1], E2_sb[:], b_cur[:], start=True, stop=True,
                             tile_position=(0, 0))
            if s < TM:
                u = crfp.tile([20, BL], F32, tag="u")
                nc.vector.scalar_tensor_tensor(
                    u[:], ps[:, 1], 0.0, esb[:, :, tb], OP.bypass, OP.mult)
                b_cur = u
            else:
                bfin = crfp.tile([20, BL], F32, tag="bf")
                nc.vector.tensor_copy(bfin[:], ps[:, 1])
                b_cur = bfin

        fin = fips.tile([1, BL], F32, tag="fin")
        v = crfp.tile([20, BL], F32, tag="v")
        nc.vector.tensor_tensor(v[:], a_cur[:], b_cur[:], OP.mult)
        nc.tensor.matmul(fin[:], ones20[:], v[:], start=True, stop=True)
        lnz = crfp.tile([1, BL], F32, tag="lnz")
        nc.scalar.activation(lnz[:], fin[:], AF.Ln)
        sc = fips.tile([1, BL], F32, tag="sc")
        nc.tensor.matmul(sc[:], ones20[:], unacc[:], start=True, stop=False)
        nc.tensor.matmul(sc[:], ones20[:], binacc[:], start=False, stop=True)
        res = crfp.tile([1, BL], F32, tag="res")
        nc.vector.scalar_tensor_tensor(res[:], lnz[:], -1.0, sc[:], OP.mult, OP.add)
        res2 = crfp.tile([1, BL], F32, tag="res2")
        nc.vector.tensor_scalar_add(res2[:], res[:], -float(T) * SIGMA)
        nc.sync.dma_start(io["out_ll"][:], res2[:])


# ---------------------------------------------------------------- host packing

def _bf(x):
    return np.ascontiguousarray(x, dtype=BFNP)


def _f32(x):
    return np.ascontiguousarray(x, dtype=np.float32)


def pack_shared(w, T):
    """Shared (replicated) weight arrays -> dict of np arrays."""
    out = {}
    convp = np.zeros((D, NPAIR * 128), np.float32)
    ws = [w["w1"], w["w2"], w["w3"], w["w4"]]  # [K, D, C]
    # channel block ch0 of conv j starts at j*C in the concat
    for p, (mb, off) in enumerate(PAIRS):
        lo, hi = mb * 128, (mb + 1) * 128
        for j, wj in enumerate(ws):
            Kj = wj.shape[0]
            pad_l = (Kj - 1) // 2
            c0, c1 = j * C, (j + 1) * C
            s, e = max(lo, c0), min(hi, c1)
            if s >= e:
                continue
            kk = off + pad_l  # tap index within this conv
            if 0 <= kk < Kj:
                convp[:, p * 128 + (s - lo): p * 128 + (e - lo)] = wj[kk][:, s - c0:e - c0]
    if FP8_CONV:
        out["convp"] = np.ascontiguousarray(convp * WSC, dtype=FP8NP)
        out["g1"] = _f32(w["ln1_g"].reshape(KD, 128).T * XSC)
        out["b1"] = _f32(w["ln1_b"].reshape(KD, 128).T * XSC)
    else:
        out["convp"] = _bf(convp)
        out["g1"] = _f32(w["ln1_g"].reshape(KD, 128).T)
        out["b1"] = _f32(w["ln1_b"].reshape(KD, 128).T)
    ln1b = np.asarray(w["ln1_b"], np.float64)
    bconv = np.concatenate([
        np.broadcast_to(w[f"b{j + 1}"], (C,)).astype(np.float64)
        + np.einsum("kdc,d->c", np.asarray(w[f"w{j + 1}"], np.float64), ln1b)
        for j in range(4)])
    out["bconv"] = _f32(bconv.reshape(6, 128).T)
    out["g2"] = _f32(w["ln2_g"].reshape(KC, 128).T * (XSC2 if FP8_XW else 1.0))
    out["b2"] = _f32(w["ln2_b"].reshape(KC, 128).T)
    # g-gate columns (last H after PERM) are scaled by 2 so the device uses
    # one sigmoid over all gates: tanh(x) = 2*sigmoid(2x) - 1
    gsc = np.ones(G4, np.float32)
    gsc[3 * H:] = 2.0
    wx_cat = np.concatenate(
        [w["wx_f"][:, PERM] * gsc, w["wx_b"][:, PERM] * gsc], axis=1)
    if FP8_XW:
        out["wx"] = np.ascontiguousarray(np.asarray(wx_cat) * WSC2, dtype=FP8NP)
    else:
        out["wx"] = _bf(wx_cat)
    wh_cat = np.concatenate(
        [w["wh_f"][:, PERM] * gsc, w["wh_b"][:, PERM] * gsc], axis=1)
    # fp8 Wh unscaled: values ~N(0,.02) land partly subnormal, which numpy
    # simulation shows costs < 5e-5 end-to-end rel err
    out["wh"] = np.ascontiguousarray(np.asarray(wh_cat), dtype=FP8NP) \
        if FP8_H else _bf(wh_cat)
    ln2b = np.asarray(w["ln2_b"], np.float64)
    sh_f = ln2b @ np.asarray(w["wx_f"], np.float64)[:, PERM]
    sh_b = ln2b @ np.asarray(w["wx_b"], np.float64)[:, PERM]
    bz = np.stack([(w["bf"][PERM] + sh_f) * gsc,
                   (w["bb"][PERM] + sh_b) * gsc]).astype(np.float32).reshape(2, MG, 128)
    out["bz"] = _f32(np.moveaxis(bz, 2, 0))  # [128, 2, MG]
    if FP8_H:
        # wd scaled x64 into fp8 range; em PSUM is then 64x -> the Exp ACT
        # de-scales via scale=1/WDSC, and the unary-score STT compensates via
        # bd*64 and onehot/64 (dump = (em' + 64 bd) * (oh/64))
        out["wd"] = np.ascontiguousarray(np.asarray(w["wd"]) * WDSC, dtype=FP8NP)
        out["bd"] = _f32(w["bd"].reshape(LBL, 1) * WDSC)
    else:
        out["wd"] = _bf(w["wd"])
        out["bd"] = _f32(w["bd"].reshape(LBL, 1))
    out["bdm"] = _f32(w["bd"].reshape(LBL, 1) - SIGMA)
    out["trans"] = _f32(w["trans"])
    out["transT"] = _f32(np.asarray(w["trans"]).T)
    return out


def pack_core(hid_a, hid_b, targets, c0, T):
    """Per-core data arrays for batch slice [c0, c0+BL)."""
    out = {}
    ha = np.asarray(hid_a[c0:c0 + BL])  # [BL, T, D_BERT]
    hb = np.asarray(hid_b[c0:c0 + BL])
    hidT = np.empty((D, BL, T), BFNP)
    hidT[:D_BERT] = ha.transpose(2, 0, 1)
    hidT[D_BERT:] = hb.transpose(2, 0, 1)
    out["hidT"] = hidT
    tg = np.asarray(targets[c0:c0 + BL])  # [BL, T] int32
    oh = np.zeros((BL, T, LBL), np.float32)
    np.put_along_axis(oh, tg[..., None], 1.0, axis=2)
    out["ohT"] = _f32(oh.transpose(2, 0, 1) * (1.0 / WDSC if FP8_H else 1.0))
    ohp = np.zeros((BL, T, LBL), BFNP)
    ohn = np.zeros((BL, T, LBL), BFNP)
    ohp[:, :T - 1] = oh[:, :T - 1]
    ohn[:, :T - 1] = oh[:, 1:]
    out["ohp"] = ohp
    out["ohn"] = ohn
    return out


# ---------------------------------------------------------------- numpy oracle

def numpy_reference(inputs, attention_mask, targets, hid_a, hid_b, ln1_g, ln1_b,
                    w1, b1, w2, b2, w3, b3, w4, b4, ln2_g, ln2_b,
                    wx_f, wh_f, bf, wx_b, wh_b, bb, wd, bd, trans):
    """Pure-numpy double-precision port of reference.py (general fallback)."""
    def ln(x, g, b):
        m = x.mean(-1, keepdims=True)
        v = ((x - m) ** 2).mean(-1, keepdims=True)
        return (x - m) / np.sqrt(v + EPS) * g + b

    def conv1d_relu(x, w, b):
        K = w.shape[0]
        pad_l = (K - 1) // 2
        Bn, Tn, Din = x.shape
        xp = np.zeros((Bn, Tn + K - 1, Din), x.dtype)
        xp[:, pad_l:pad_l + Tn] = x
        y = np.zeros((Bn, Tn, w.shape[2]), x.dtype)
        for k in range(K):
            y += xp[:, k:k + Tn] @ w[k]
        return np.maximum(y + b, 0.0)

    def sig(x):
        return 1.0 / (1.0 + np.exp(-x))

    def lstm(x, mask, Wx, Wh, bias, reverse):
        Bn, Tn, _ = x.shape
        Hn = Wh.shape[0]
        h = np.zeros((Bn, Hn), x.dtype)
        c = np.zeros((Bn, Hn), x.dtype)
        op = np.zeros((Bn, Hn), x.dtype)
        ys = np.zeros((Bn, Tn, Hn), x.dtype)
        order = range(Tn - 1, -1, -1) if reverse else range(Tn)
        for t in order:
            z = x[:, t] @ Wx + h @ Wh + bias
            i, f, g, o = np.split(z, 4, axis=-1)
            i, f, o = sig(i), sig(f), sig(o)
            cn = f * c + i * np.tanh(g)
            hn = o * np.tanh(cn)
            m = mask[:, t][:, None]
            h = np.where(m, hn, h)
            c = np.where(m, cn, c)
            op = np.where(m, hn, op)
            ys[:, t] = op
        return ys

    x = np.concatenate([np.asarray(hid_a, np.float64), np.asarray(hid_b, np.float64)], axis=-1)
    x = ln(x, np.asarray(ln1_g, np.float64), np.asarray(ln1_b, np.float64))
    conv = np.concatenate([
        conv1d_relu(x, np.asarray(w1, np.float64), b1),
        conv1d_relu(x, np.asarray(w2, np.float64), b2),
        conv1d_relu(x, np.asarray(w3, np.float64), b3),
        conv1d_relu(x, np.asarray(w4, np.float64), b4)], axis=-1)
    conv = ln(conv, np.asarray(ln2_g, np.float64), np.asarray(ln2_b, np.float64))
    mask = np.asarray(attention_mask) != 0
    hf = lstm(conv, mask, np.asarray(wx_f, np.float64), np.asarray(wh_f, np.float64),
              np.asarray(bf, np.float64), False)
    hbk = lstm(conv, mask, np.asarray(wx_b, np.float64), np.asarray(wh_b, np.float64),
               np.asarray(bb, np.float64), True)
    h = np.concatenate([hf, hbk], axis=-1)
    logits = h @ np.asarray(wd, np.float64) + np.asarray(bd, np.float64)
    seq_len = (np.asarray(inputs) != 0).astype(np.int64).sum(1)
    Bn, Tn, L = logits.shape
    tg = np.asarray(targets)
    valid = np.arange(Tn)[None, :] < seq_len[:, None]
    unary = np.take_along_axis(logits, tg[..., None], axis=2)[..., 0]
    unary_score = np.where(valid, unary, 0.0).sum(1)
    pair = np.asarray(trans, np.float64)[tg[:, :-1], tg[:, 1:]]
    binary_score = np.where(valid[:, 1:], pair, 0.0).sum(1)
    alpha = logits[:, 0]
    tr = np.asarray(trans, np.float64)
    for t in range(1, Tn):
        nxt = alpha[:, :, None] + tr[None, :, :]
        mx = nxt.max(1)
        nxt = np.log(np.exp(nxt - mx[:, None, :]).sum(1)) + mx + logits[:, t]
        alpha = np.where(valid[:, t][:, None], nxt, alpha)
    mx = alpha.max(1)
    log_norm = np.log(np.exp(alpha - mx[:, None]).sum(1)) + mx
    return (unary_score + binary_score - log_norm).astype(np.float32)


# ---------------------------------------------------------------- program build

_CACHE = {}


def build_program(T=T_FULL, TCH=64):
    key = (T, TCH)
    if key in _CACHE:
        return _CACHE[key]
    nc = bacc.Bacc("TRN2", target_bir_lowering=False, debug=False,
                   enable_asserts=False, num_devices=NCORE)
    io = {}

    def din(name, shape, dt):
        io[name] = nc.dram_tensor(name, shape, dt, kind="ExternalInput").ap()

    din("hidT", [D, BL, T], BF16)
    din("convp", [D, NPAIR * 128], FP8 if FP8_CONV else BF16)
    din("bconv", [128, 6], F32)
    din("g1", [128, KD], F32)
    din("b1", [128, KD], F32)
    din("g2", [128, KC], F32)
    din("b2", [128, KC], F32)
    din("wx", [C4, 2 * G4], FP8 if FP8_XW else BF16)
    din("wh", [H, 2 * G4], FP8 if FP8_H else BF16)
    din("bz", [128, 2, MG], F32)
    din("wd", [2 * H, LBL], FP8 if FP8_H else BF16)
    din("bd", [LBL, 1], F32)
    din("bdm", [LBL, 1], F32)
    din("trans", [LBL, LBL], F32)
    din("transT", [LBL, LBL], F32)
    din("ohT", [LBL, BL, T], F32)
    din("ohp", [BL, T, LBL], BF16)
    din("ohn", [BL, T, LBL], BF16)
    io["out_ll"] = nc.dram_tensor("out_ll", [1, BL], F32, kind="ExternalOutput").ap()

    with tile.TileContext(nc) as tc:
        _emit(tc, io, T, TCH)
    nc.compile()
    _CACHE[key] = nc
    return nc


# ---------------------------------------------------------------- entry point

TRACE = False          # set True (e.g. from test.py) to capture an NTFF profile
LAST_RESULTS = None    # BassKernelResults of the most recent run


def kernel(**inputs):
    global LAST_RESULTS
    inputs = {k: np.asarray(v) for k, v in inputs.items()}
    if (inputs["inputs"] == 0).any() or (inputs["attention_mask"] == 0).any():
        # out-of-distribution (masked) input: exact host fallback
        return numpy_reference(**inputs)

    from concourse.bass_utils import run_bass_kernel_spmd

    T = inputs["inputs"].shape[1]
    nc = build_program(T=T)
    shared = pack_shared(inputs, T)
    in_maps = []
    for core in range(NCORE):
        m = dict(shared)
        m.update(pack_core(inputs["hid_a"], inputs["hid_b"], inputs["targets"],
                           core * BL, T))
        in_maps.append(m)
    res = run_bass_kernel_spmd(nc, in_maps, core_ids=list(range(NCORE)), trace=TRACE)
    LAST_RESULTS = res
    out = np.concatenate([res.results[c]["out_ll"][0] for c in range(NCORE)])
    return out.astype(np.float32)


if __name__ == "__main__":
    print("kernel module ok")



# revision 47
# speedup vs baseline: 1.0615x; 1.0615x over previous
"""Trainium2 Bass kernel for BiLSTM-CRF log-likelihood.

Pipeline (per core, pure data-parallel over batch: 8 of 64 sequences/core):
  concat(hid_a,hid_b) -> LN -> 4x conv1d(k=1..4)+relu -> LN -> BiLSTM(256)
  -> dense(20) -> CRF log-likelihood  -> [B] scores.

Device layout is feature-major ("transposed"): features on SBUF partitions,
(batch, time) on the free axis.  All big matmuls run in bf16 with fp32 PSUM
accumulation.  The CRF forward recursion runs in exp-space:
    a_t = (a_{t-1} @ exp(trans)) * exp(emit_t + bd - sigma)
with a constant per-step rescale exp(-sigma) keeping a ~ O(1), so
log Z = log(sum a_{T-1}) + T*sigma.  LSTM recurrence keeps Wh stationary
(8 m-tiles x 2 k-tiles of 128), batch in the moving free dim.

The attention mask is all-ones and no token id is 0 under the problem's input
distribution (randint low=1, mask fill=ones); the device kernel assumes that
and a host-side numpy fallback handles any other input.
"""

import os
import sys
from contextlib import ExitStack

import numpy as np

for _p in ("/opt/trn_rl_repo", "/root/.axon_site/_ro/trn_rl_repo"):
    if os.path.isdir(_p) and _p not in sys.path:
        sys.path.append(_p)

import ml_dtypes  # noqa: E402

import concourse.bass as bass  # noqa: E402
import concourse.tile as tile  # noqa: E402
from concourse import bacc, mybir  # noqa: E402
from concourse._compat import with_exitstack  # noqa: E402
from concourse.alu_op_type import AluOpType  # noqa: E402
from concourse.bass import ds, ts  # noqa: E402

F32 = mybir.dt.float32
BF16 = mybir.dt.bfloat16
FP8 = mybir.dt.float8e4
DR = mybir.MatmulPerfMode.DoubleRow
FP8NP = ml_dtypes.float8_e4m3
AF = mybir.ActivationFunctionType
OP = AluOpType
BFNP = ml_dtypes.bfloat16

# problem dims
B, T_FULL, D_BERT, LBL, H = 64, 512, 768, 20, 256
D = 2 * D_BERT            # 1536, LN1/conv input features
C = 192
C4 = 4 * C                # 768, conv concat channels
G4 = 4 * H                # 1024, lstm gate width
NCORE = 8
BL = B // NCORE           # 8 sequences per core
KD = D // 128             # 12
KC = C4 // 128            # 6
MG = G4 // 128            # 8
KH = H // 128             # 2
KW = (2 * H) // 128       # 4 (dense k-tiles)
SIGMA = 3.0
EPS = 1e-5

# conv taps, grouped by time offset.  TF/XLA SAME padding:
# K=1 -> {0}; K=2 -> {0,+1}; K=3 -> {-1,0,+1}; K=4 -> {-1,0,+1,+2}
# concat channel blocks: conv1 0:192, conv2 192:384, conv3 384:576, conv4 576:768
# 128-wide m-blocks and which offsets are active in each:
ACTIVE = {0: [0], 1: [0, 1], 2: [0, 1], 3: [-1, 0, 1], 4: [-1, 0, 1, 2], 5: [-1, 0, 1, 2]}
PAIRS = [(mb, off) for mb in range(6) for off in ACTIVE[mb]]  # 16 (mb,off) pairs
NPAIR = len(PAIRS)
# gate reorder: keras order i,f,g,o -> device order i,f,o,g with the g block
# last and pre-scaled by 2: one ACT Sigmoid covers the whole z tile and the
# DVE recovers tanh(g) = 2*sigmoid(2g) - 1 with a cheap bf16 tensor_scalar
PERM = np.r_[0:H, H:2 * H, 3 * H:4 * H, 2 * H:3 * H]
FP8_CONV = True            # conv1d matmuls in fp8e4m3 DoubleRow (k=256/instr)
XSC = 8.0                  # LN1 output pre-scale (fp8 dynamic range)
WSC = 64.0                 # conv weight pre-scale (fp8 dynamic range)
FP8_XW = True              # xW matmuls in fp8e4m3 DoubleRow
XSC2 = 8.0                 # LN2 output pre-scale
WSC2 = 64.0                # wx weight pre-scale
FP8_H = False              # fp8 DoubleRow Wh measured WORSE (FD=8 kills FWL: LDW 213ns
                           # vs 26ns dominates the on-chain burst; +436us) — keep bf16
TGACT = False              # tanh(g) via a 2nd ACT measured WORSE (+220us): 6 ACTs/step
                           # saturate the Scalar queue and lengthen the chain — keep the
                           # DVE 2s-1 conversion (4 ACTs/step)
NHEAT = 0                  # HAM-heater matmuls measured WORSE both ways: single PSUM
                           # tile WAW-serializes (+246us); rotating tiles still +107us
                           # (PE-FIFO insertion ahead of the real gate burst) — off
WDSC = 64.0                # wd weight pre-scale (fp8), compensated via oht/bd host packing
BLP = 16                   # hbuf batch-axis padding so the k-pair stride is 16B


# ---------------------------------------------------------------- device build

@with_exitstack
def _emit(ctx, tc, io, T, TCH):
    """Emit the full program. io: dict name -> dram AP."""
    nc = tc.nc
    TP = T + 3  # padded time axis (1 left, 2 right) for conv taps
    TP8 = T + 16  # fp8 pair-tile time axis: 16B-aligned stride for DoubleRow APs

    per = ctx.enter_context(tc.tile_pool(name="persist", bufs=1))

    # --- persistent constants / weights -> SBUF
    ones1b = per.tile([128, 1], BF16)
    nc.any.memset(ones1b[:], 1.0)
    ones1f = per.tile([128, 1], F32)
    nc.any.memset(ones1f[:], 1.0)
    onesrowf = per.tile([1, 128], F32)
    nc.any.memset(onesrowf[:], 1.0)
    ones20 = per.tile([20, 1], F32)
    nc.any.memset(ones20[:], 1.0)
    epscol = per.tile([1, 1], F32)
    nc.any.memset(epscol[:], EPS)

    if FP8_H:
        wh_sb = per.tile([128, KH, 2 * G4], FP8)
        for k in range(KH):
            nc.sync.dma_start(wh_sb[:, k], io["wh"][ds(k * 128, 128), :])
        wd_sb = per.tile([128, KW, LBL], FP8)
        for k in range(KW):
            nc.sync.dma_start(wd_sb[:, k], io["wd"][ds(k * 128, 128), :])
    else:
        wh_sb = per.tile([128, KH, 2 * G4], BF16)
        nc.sync.dma_start(wh_sb[:], io["wh"].rearrange("(ko p) m -> p ko m", p=128))
        wd_sb = per.tile([128, KW, LBL], BF16)
        nc.sync.dma_start(wd_sb[:], io["wd"].rearrange("(ko p) m -> p ko m", p=128))
    bz_sb = per.tile([128, 2, MG], F32)
    nc.sync.dma_start(bz_sb[:], io["bz"])
    bd_sb = per.tile([20, 1], F32)
    nc.sync.dma_start(bd_sb[:], io["bd"])
    bdm_sb = per.tile([20, 1], F32)
    nc.sync.dma_start(bdm_sb[:], io["bdm"])
    trans_sb = per.tile([20, 20], F32)
    nc.sync.dma_start(trans_sb[:], io["trans"])
    transT_sb = per.tile([20, 20], F32)
    nc.sync.dma_start(transT_sb[:], io["transT"])

    # persistent state across phases (time-major so the per-step h write is a
    # contiguous [128, KH*BL] slice -> DVE 2x mode).  With FP8_H the batch
    # axis pads to 16 so the k-pair axis stride is 16B (DoubleRow AP rule).
    HDT = FP8 if FP8_H else BF16
    HBL = BLP if FP8_H else BL
    hbuf_f = per.tile([128, T + 1, KH, HBL], HDT)
    hbuf_b = per.tile([128, T + 1, KH, HBL], HDT)
    esb = per.tile([20, BL, T], BF16)      # exp(emit + bd - sigma)
    unacc = per.tile([20, BL], F32)
    binacc = per.tile([20, BL], F32)

    dram = ctx.enter_context(tc.tile_pool(name="dram", bufs=1, space="DRAM"))
    xwt = dram.tile([2, G4, BL, T], BF16)  # x@Wx + bias, gate-major, per dir

    # =================================================== phase A: LN1/conv/LN2/xW
    with ExitStack() as pa:
        wpool = pa.enter_context(tc.tile_pool(name="wconv", bufs=1))
        if FP8_CONV:
            convw = wpool.tile([128, KD // 2, 2, NPAIR * 128], FP8)
            for k in range(KD):
                nc.sync.dma_start(convw[:, k // 2, k % 2],
                                  io["convp"][ds(k * 128, 128), :])
        else:
            convw = wpool.tile([128, KD, NPAIR * 128], BF16)
            for k in range(KD):
                nc.sync.dma_start(convw[:, k], io["convp"][ds(k * 128, 128), :])
        if FP8_XW:
            wx_sb = wpool.tile([128, KC // 2, 2, 2 * G4], FP8)
            for k in range(KC):
                nc.sync.dma_start(wx_sb[:, k // 2, k % 2],
                                  io["wx"][ds(k * 128, 128), :])
        else:
            wx_sb = wpool.tile([128, KC, 2 * G4], BF16)
            nc.sync.dma_start(wx_sb[:], io["wx"].rearrange("(ko p) m -> p ko m", p=128))
        g1_sb = wpool.tile([128, KD], F32)
        nc.sync.dma_start(g1_sb[:], io["g1"])
        g2_sb = wpool.tile([128, KC], F32)
        nc.sync.dma_start(g2_sb[:], io["g2"])
        bcv_sb = wpool.tile([128, 6], F32)
        nc.sync.dma_start(bcv_sb[:], io["bconv"])

        sqp = pa.enter_context(tc.tile_pool(name="sq", bufs=6))
        tmpp = pa.enter_context(tc.tile_pool(name="lntmp", bufs=6))
        xpp = pa.enter_context(tc.tile_pool(name="xp", bufs=2 * KD))
        xp8p = pa.enter_context(tc.tile_pool(name="xp8", bufs=KD))
        cvp = pa.enter_context(tc.tile_pool(name="cvr", bufs=12))
        cv8p = pa.enter_context(tc.tile_pool(name="cv8", bufs=6))
        stgp = pa.enter_context(tc.tile_pool(name="stage", bufs=3))
        smallp = pa.enter_context(tc.tile_pool(name="lnsmall", bufs=2))
        sumps = pa.enter_context(tc.tile_pool(name="sums", bufs=1, space="PSUM"))
        bcps = pa.enter_context(tc.tile_pool(name="bcast", bufs=1, space="PSUM"))
        cvps = pa.enter_context(tc.tile_pool(name="cvps", bufs=2, space="PSUM"))
        xwps = pa.enter_context(tc.tile_pool(name="xwps", bufs=2, space="PSUM"))

        def layer_norm_T(xin, nk, gg, out_of):
            """Feature-major layernorm over nk*128 features, in-place capable.
            xin: list of [128, T] APs; out_of(k) -> output AP (may alias xin[k])."""
            s1 = sumps.tile([1, T], F32, tag="s1")
            s2 = sumps.tile([1, T], F32, tag="s2")
            for k in range(nk):
                nc.tensor.matmul(s1[:], ones1b[:], xin[k], start=(k == 0), stop=(k == nk - 1))
            for k in range(nk):
                sq = sqp.tile([128, T], BF16, tag="sq")
                # x*x on DVE (bf16 2x) instead of ACT Square: ~330ns vs ~720ns,
                # and keeps the Scalar engine free for the relu/LN broadcasts
                nc.vector.tensor_tensor(sq[:], xin[k], xin[k], OP.mult)
                nc.tensor.matmul(s2[:], ones1b[:], sq[:], start=(k == 0), stop=(k == nk - 1))
            nf = float(nk * 128)
            mu = smallp.tile([1, T], F32, tag="mu")
            nc.scalar.mul(mu[:], s1[:], 1.0 / nf)
            mu2 = smallp.tile([1, T], F32, tag="mu2")
            nc.vector.tensor_tensor(mu2[:], mu[:], mu[:], OP.mult)
            varr = smallp.tile([1, T], F32, tag="varr")
            nc.vector.scalar_tensor_tensor(varr[:], s2[:], 1.0 / nf, mu2[:], OP.mult, OP.subtract)
            # rsqrt via exp(-0.5*ln(v+eps)): Ln/Exp share one ACT table with
            # Square/Relu/Identity, and this avoids the slow DVE reciprocal
            lnv = smallp.tile([1, T], F32, tag="lnv")
            nc.scalar.activation(lnv[:], varr[:], AF.Ln, bias=epscol[0:1, 0:1])
            rr = smallp.tile([1, T], F32, tag="rr")
            nc.scalar.activation(rr[:], lnv[:], AF.Exp, scale=-0.5)
            mub_ps = bcps.tile([128, T], F32, tag="mub")
            nc.tensor.matmul(mub_ps[:], onesrowf[:], mu[:], start=True, stop=True)
            rb_ps = bcps.tile([128, T], F32, tag="rb")
            nc.tensor.matmul(rb_ps[:], onesrowf[:], rr[:], start=True, stop=True)
            # bf16 SBUF copies of the broadcasts: the per-k DVE ops below then
            # run in 2x mode (all-SBUF, all-16-bit operands)
            mub = tmpp.tile([128, T], BF16, tag="mubs")
            nc.scalar.activation(mub[:], mub_ps[:], AF.Identity)
            rb = tmpp.tile([128, T], BF16, tag="rbs")
            nc.scalar.activation(rb[:], rb_ps[:], AF.Identity)
            for k in range(nk):
                t1 = tmpp.tile([128, T], BF16, tag="lnt")
                nc.vector.tensor_tensor(t1[:], xin[k], mub[:], OP.subtract)
                nc.vector.tensor_tensor(t1[:], t1[:], rb[:], OP.mult)
                nc.vector.tensor_scalar_mul(out_of(k), t1[:], gg[:, k:k + 1])

        def stage_load(b):
            xp = []
            for k in range(KD):
                t = xpp.tile([128, TP], BF16, tag="xp")
                nc.sync.dma_start(t[:, 1:T + 1], io["hidT"][ts(k, 128), b, :])
                xp.append(t)
            return xp

        def stage_ln1(xp):
            if FP8_CONV:
                xp8 = []
                for j in range(KD // 2):
                    t8 = xp8p.tile([128, 2, TP8], FP8, tag="xp8")
                    nc.any.memset(t8[:, :, 0:1], 0.0)
                    nc.any.memset(t8[:, :, T + 1:TP8], 0.0)
                    xp8.append(t8)
                layer_norm_T([xp[k][:, 1:T + 1] for k in range(KD)],
                             KD, g1_sb,
                             lambda k: xp8[k // 2][:, k % 2, 1:T + 1])
                return xp8
            for k in range(KD):
                nc.any.memset(xp[k][:, 0:1], 0.0)
                nc.any.memset(xp[k][:, T + 1:TP], 0.0)
            layer_norm_T([xp[k][:, 1:T + 1] for k in range(KD)],
                         KD, g1_sb, lambda k: xp[k][:, 1:T + 1])
            return xp

        def stage_conv(xp_or_xp8):
            xp8 = xp = xp_or_xp8
            cvr = []
            for mb in range(6):
                cv = cvps.tile([128, T], F32, tag="cv")
                mms = [(p, off) for p, (mb2, off) in enumerate(PAIRS) if mb2 == mb]
                if FP8_CONV:
                    n_mm = len(mms) * (KD // 2)
                    i = 0
                    for p, off in mms:
                        for kj in range(KD // 2):
                            nc.tensor.matmul(
                                cv[:], convw[:, kj, :, ds(p * 128, 128)],
                                xp8[kj][:, :, 1 + off: 1 + off + T],
                                start=(i == 0), stop=(i == n_mm - 1),
                                perf_mode=DR)
                            i += 1
                    relu_scale = 1.0 / (XSC * WSC)
                else:
                    n_mm = len(mms) * KD
                    i = 0
                    for p, off in mms:
                        for k in range(KD):
                            nc.tensor.matmul(
                                cv[:], convw[:, k, ds(p * 128, 128)],
                                xp[k][:, 1 + off: 1 + off + T],
                                start=(i == 0), stop=(i == n_mm - 1))
                            i += 1
                    relu_scale = 1.0
                out = cvp.tile([128, T], BF16, tag="cvr")
                nc.scalar.activation(out[:], cv[:], AF.Relu,
                                     bias=bcv_sb[:, mb:mb + 1], scale=relu_scale)
                cvr.append(out)
            return cvr

        def stage_ln2(cvr):
            if FP8_XW:
                cv8 = []
                for j in range(KC // 2):
                    t8 = cv8p.tile([128, 2, T], FP8, tag="cv8")
                    cv8.append(t8)
                layer_norm_T([cvr[k][:] for k in range(KC)],
                             KC, g2_sb, lambda k: cv8[k // 2][:, k % 2, :])
                return cv8
            layer_norm_T([cvr[k][:] for k in range(KC)],
                         KC, g2_sb, lambda k: cvr[k][:])
            return cvr

        def stage_xw(b, cvx):
            for d in range(2):
                for m in range(MG):
                    xw = xwps.tile([128, T], F32, tag="xw")
                    if FP8_XW:
                        for kj in range(KC // 2):
                            nc.tensor.matmul(
                                xw[:], wx_sb[:, kj, :, ds(d * G4 + m * 128, 128)],
                                cvx[kj][:],
                                start=(kj == 0), stop=(kj == KC // 2 - 1),
                                perf_mode=DR)
                        xw_scale = 1.0 / (XSC2 * WSC2)
                    else:
                        for k in range(KC):
                            nc.tensor.matmul(
                                xw[:], wx_sb[:, k, ds(d * G4 + m * 128, 128)], cvx[k][:],
                                start=(k == 0), stop=(k == KC - 1))
                        xw_scale = 1.0
                    stg = stgp.tile([128, T], BF16, tag="stg")
                    # de-scale + bias-add + downcast in one ACT (Scalar is idle here)
                    nc.scalar.activation(stg[:], xw[:], AF.Identity,
                                         bias=bz_sb[:, d, m:m + 1], scale=xw_scale)
                    nc.sync.dma_start(xwt[d, ds(m * 128, 128), b, :], stg[:])

        # sequential per-sequence emission (a cross-sequence software pipeline
        # was measured WORSE: +380us)
        for b in range(BL):
            xp = stage_load(b)
            xp8 = stage_ln1(xp)
            cvr = stage_conv(xp8)
            cvx = stage_ln2(cvr)
            stage_xw(b, cvx)

    # =================================================== phase B: BiLSTM
    with ExitStack() as pb:
        xchp = pb.enter_context(tc.tile_pool(name="xch", bufs=2))
        zsp = pb.enter_context(tc.tile_pool(name="zs", bufs=2))
        gsp = pb.enter_context(tc.tile_pool(name="gs", bufs=4))
        ctp = pb.enter_context(tc.tile_pool(name="ct", bufs=4))
        ttp = pb.enter_context(tc.tile_pool(name="tt", bufs=6))
        zps = pb.enter_context(tc.tile_pool(name="zps", bufs=2, space="PSUM"))
        htps = pb.enter_context(tc.tile_pool(name="heat", bufs=1, space="PSUM"))

        nc.any.memset(hbuf_f[:, 0], 0.0)
        nc.any.memset(hbuf_b[:, T], 0.0)
        c_cur = [None, None]
        for d in range(2):
            cz = ctp.tile([128, KH, BL], BF16, tag=f"c{d}")
            nc.any.memset(cz[:], 0.0)
            c_cur[d] = cz

        def dma_chunk(t, d, sl):
            for m in range(MG):
                nc.sync.dma_start(t[:, m], xwt[d, ds(m * 128, 128), :, sl])

        nch = T // TCH
        xch_cur = [None, None]
        for d in range(2):
            t = xchp.tile([128, MG, BL, TCH], BF16, tag=f"xch{d}")
            dma_chunk(t, d, ds(0, TCH) if d == 0 else ds(T - TCH, TCH))
            xch_cur[d] = t

        # identity to fold the precomputed xW step-slice into PSUM on the PE,
        # so ACT reads z straight from PSUM (no VE add on the critical chain)
        from concourse.masks import make_identity
        ident = ctp.tile([128, 128], BF16, tag="ident")
        make_identity(nc, ident[:])

        hb = [hbuf_f, hbuf_b]
        # gate column order is i,f,o,g (PERM): the xW step-slice folds into the
        # z PSUM tile with ONE identity matmul (free=64) per dir, then per-m
        # Wh matmuls accumulate; one ACT Sigmoid covers i,f,o and one ACT Tanh
        # covers g.  The two directions' chains interleave so each hides in
        # the other's latency gaps.
        heat_ps = []
        if NHEAT:
            # one PSUM tile per heater slot: no WAW chain between consecutive
            # heater MMs (a single tile serialized them at ~270ns each)
            for j in range(NHEAT):
                heat_t = htps.tile([128, 256], F32, tag=f"heat{j}")
                heat_ps.append(heat_t)
        for step in range(T):
            if NHEAT and step > 0:
                # HAM heater: keep the PE clock-gate warm through the chain
                # gap (these run while the PE waits on the h-writes)
                for j in range(NHEAT):
                    nc.tensor.matmul(heat_ps[j][:], ident[:], wh_sb[:, 0, 0:256],
                                     start=True, stop=True)
            zp2, gs2, tg2 = [None, None], [None, None], [None, None]
            for d in range(2):
                tt = step if d == 0 else T - 1 - step
                ci = step // TCH
                tl = step % TCH
                if tl == 0 and ci + 1 < nch:
                    nxt = xchp.tile([128, MG, BL, TCH], BF16, tag=f"xch{d}")
                    dma_chunk(nxt, d,
                              ds((ci + 1) * TCH, TCH) if d == 0 else ds(T - (ci + 2) * TCH, TCH))
                    xch_cur[d] = (xch_cur[d], nxt)
                xc = xch_cur[d][0] if isinstance(xch_cur[d], tuple) else xch_cur[d]
                tcl = tl if d == 0 else TCH - 1 - tl
                hcol = tt if d == 0 else tt + 1
                zp = zps.tile([128, MG * BL], F32, tag=f"zp{d}")
                nc.tensor.matmul(zp[:], ident[:], xc[:, :, :, tcl],
                                 start=True, stop=False)
                if FP8_H:
                    # DoubleRow folds both k-tiles into one MM: 8 on-chain
                    # matmuls per dir-step instead of 16
                    for m in range(MG):
                        nc.tensor.matmul(
                            zp[:, ds(m * BL, BL)],
                            wh_sb[:, :, ds(d * G4 + m * 128, 128)],
                            hb[d][:, hcol, :, 0:BL],
                            start=False, stop=(m == MG - 1), perf_mode=DR)
                else:
                    for m in range(MG):
                        for k in range(KH):
                            nc.tensor.matmul(
                                zp[:, ds(m * BL, BL)],
                                wh_sb[:, k, ds(d * G4 + m * 128, 128)],
                                hb[d][:, hcol, k, :],
                                start=False,
                                stop=(m == MG - 1 and k == KH - 1))
                zp2[d] = zp
            for d in range(2):
                if TGACT:
                    # tanh(g) straight off the z PSUM g-slice (g pre-scaled x2
                    # in the packing, so scale=0.5 recovers tanh(g)); emitted
                    # before the sigmoid so both land by the time t1 needs them
                    tgt = gsp.tile([128, KH, BL], BF16, tag=f"tga{d}")
                    nc.scalar.activation(
                        tgt[:],
                        zp2[d][:, ds(6 * BL, KH * BL)].rearrange("p (m b) -> p m b", m=KH),
                        AF.Tanh, scale=0.5)
                    tg2[d] = tgt
                    gs = gsp.tile([128, 6, BL], BF16, tag=f"gs{d}")
                    nc.scalar.activation(gs[:],
                                         zp2[d][:, 0:6 * BL].rearrange("p (m b) -> p m b", m=6),
                                         AF.Sigmoid)
                else:
                    gs = gsp.tile([128, MG, BL], BF16, tag=f"gs{d}")
                    nc.scalar.activation(gs[:],
                                         zp2[d][:].rearrange("p (m b) -> p m b", m=MG),
                                         AF.Sigmoid)
                gs2[d] = gs
            cn2 = [None, None]
            for d in range(2):
                gs = gs2[d]  # i=0:2, f=2:4, o=4:6 (+ g=6:8 as sigmoid(2g) if not TGACT)
                if TGACT:
                    tg = tg2[d]
                else:
                    tg = ttp.tile([128, KH, BL], BF16, tag=f"tg{d}")
                    nc.vector.tensor_scalar(tg[:], gs[:, 6:8, :], 2.0, -1.0, OP.mult, OP.add)
                t2 = ttp.tile([128, KH, BL], BF16, tag=f"t2{d}")
                nc.vector.scalar_tensor_tensor(t2[:], c_cur[d][:], 0.0, gs[:, 2:4, :], OP.bypass, OP.mult)
                t1 = ttp.tile([128, KH, BL], BF16, tag=f"t1{d}")
                nc.vector.scalar_tensor_tensor(t1[:], gs[:, 0:2, :], 0.0, tg[:], OP.bypass, OP.mult)
                cn = ctp.tile([128, KH, BL], BF16, tag=f"c{d}")
                nc.vector.tensor_tensor(cn[:], t1[:], t2[:], OP.add)
                c_cur[d] = cn
                thc = ttp.tile([128, KH, BL], BF16, tag=f"thc{d}")
                nc.scalar.activation(thc[:], cn[:], AF.Tanh)
                cn2[d] = thc
            for d in range(2):
                tt = step if d == 0 else T - 1 - step
                wcol = tt + 1 if d == 0 else tt
                nc.vector.scalar_tensor_tensor(
                    hb[d][:, wcol, :, 0:BL], gs2[d][:, 4:6, :], 0.0, cn2[d][:],
                    OP.bypass, OP.mult)
                tl = step % TCH
                if tl == TCH - 1 and isinstance(xch_cur[d], tuple):
                    xch_cur[d] = xch_cur[d][1]

    # =================================================== phase C: logits + CRF
    with ExitStack() as pc:
        ohtp = pc.enter_context(tc.tile_pool(name="oht", bufs=2))
        ohkp = pc.enter_context(tc.tile_pool(name="ohk", bufs=8))
        dmp = pc.enter_context(tc.tile_pool(name="dump", bufs=2))
        crfp = pc.enter_context(tc.tile_pool(name="crf", bufs=4))
        emps = pc.enter_context(tc.tile_pool(name="emps", bufs=2, space="PSUM"))
        cbps = pc.enter_context(tc.tile_pool(name="cbps", bufs=1, space="PSUM"))
        apps = pc.enter_context(tc.tile_pool(name="apps", bufs=3, space="PSUM"))
        fips = pc.enter_context(tc.tile_pool(name="fips", bufs=1, space="PSUM"))

        # k-tiles over time for the bigram matmuls (partial tile for small T)
        kt_sizes = [128] * (T // 128) + ([T % 128] if T % 128 else [])
        # prefetch all one-hot tensors up front so the bigram work (emitted
        # interleaved into the scan below) never waits on DMA
        oht_all, ohp_all, ohn_all = [], [], []
        for b in range(BL):
            oht = ohtp.tile([20, T], F32, tag=f"oht{b}")
            nc.sync.dma_start(oht[:], io["ohT"][:, b, :])
            oht_all.append(oht)
            ohp_t = ohkp.tile([128, len(kt_sizes), 20], BF16, tag=f"ohp{b}")
            ohn_t = ohkp.tile([128, len(kt_sizes), 20], BF16, tag=f"ohn{b}")
            for k, ksz in enumerate(kt_sizes):
                nc.sync.dma_start(ohp_t[:ksz, k], io["ohp"][b, ds(k * 128, ksz), :])
                nc.sync.dma_start(ohn_t[:ksz, k], io["ohn"][b, ds(k * 128, ksz), :])
            ohp_all.append(ohp_t)
            ohn_all.append(ohn_t)
        for b in range(BL):
            em = emps.tile([20, T], F32, tag="em")
            for k in range(KW):
                rhs = (hbuf_f[:, 1:T + 1, k, b] if k < KH
                       else hbuf_b[:, 0:T, k - KH, b])
                nc.tensor.matmul(em[:], wd_sb[:, k, :], rhs, start=(k == 0), stop=(k == KW - 1))
            nc.scalar.activation(esb[:, b, :], em[:], AF.Exp, bias=bdm_sb[:, 0:1],
                                 scale=(1.0 / WDSC if FP8_H else 1.0))
            dump = dmp.tile([20, T], F32, tag="dump")
            nc.vector.scalar_tensor_tensor(
                dump[:], em[:], bd_sb[:, 0:1], oht_all[b][:], OP.add, OP.mult,
                accum_out=unacc[:, b:b + 1])
            cb = cbps.tile([20, 20], F32, tag="cb")
            for k, ksz in enumerate(kt_sizes):
                nc.tensor.matmul(cb[:], ohp_all[b][:ksz, k], ohn_all[b][:ksz, k],
                                 start=(k == 0), stop=(k == len(kt_sizes) - 1))
            dump2 = dmp.tile([20, 20], F32, tag="dump2")
            nc.vector.scalar_tensor_tensor(
                dump2[:], cb[:], 0.0, trans_sb[:], OP.bypass, OP.mult,
                accum_out=binacc[:, b:b + 1])

        # forward alpha and backward beta exp-space scans meet in the middle:
        # alpha_t = (E^T a_{t-1}) * e_t climbs t=1..TM-1, beta_t = E (e_{t+1} *
        # beta_{t+1}) descends t=T-2..TM-1, then Z = sum_j alpha[j]*beta[j].
        # 4 independent PE->VE->PE chains (2 batch halves x alpha/beta) hide
        # each other's latency; e-factor count stays T so the -T*SIGMA
        # correction is unchanged.
        TM = T // 2
        E_sb = crfp.tile([20, 20], F32, tag="E")
        nc.scalar.activation(E_sb[:], trans_sb[:], AF.Exp)
        E2_sb = crfp.tile([20, 20], F32, tag="E2")
        nc.scalar.activation(E2_sb[:], transT_sb[:], AF.Exp)
        # full batch per chain: the alpha and beta scans are already two
        # independent PE->VE->PE chains that hide each other's latency, and
        # the scan is DVE-throughput-bound, so fewer/wider DVE ops win
        a_cur = crfp.tile([20, BL], F32, tag="a0")
        nc.vector.tensor_copy(a_cur[:], esb[:, :, 0])
        b_cur = crfp.tile([20, BL], F32, tag="u0")
        nc.vector.tensor_copy(b_cur[:], esb[:, :, T - 1])
        for s in range(1, TM + 1):
            ps = apps.tile([20, 2, BL], F32, tag="scanps")
            if s <= TM - 1:
                # tile_position pins the 20x20 MM to one 32x32 subarray so the
                # systolic drain (on the scan's critical cycle) is ~32 rows
                nc.tensor.matmul(ps[:, 0], E_sb[:], a_cur[:], start=True, stop=True,
                                 tile_position=(0, 0))
                a_nxt = crfp.tile([20, BL], F32, tag="a")
                nc.vector.scalar_tensor_tensor(
                    a_nxt[:], ps[:, 0], 0.0, esb[:, :, s], OP.bypass, OP.mult)
                a_cur = a_nxt
            tb = T - 1 - s  # beta index produced this slot: 510 .. 255
            nc.tensor.matmul(ps[:, 1], E2_sb[:], b_cur[:], start=True, stop=True,
                             tile_position=(0, 0))
            if s < TM:
                u = crfp.tile([20, BL], F32, tag="u")
                nc.vector.scalar_tensor_tensor(
                    u[:], ps[:, 1], 0.0, esb[:, :, tb], OP.bypass, OP.mult)
                b_cur = u
            else:
                bfin = crfp.tile([20, BL], F32, tag="bf")
                nc.vector.tensor_copy(bfin[:], ps[:, 1])
                b_cur = bfin

        fin = fips.tile([1, BL], F32, tag="fin")
        v = crfp.tile([20, BL], F32, tag="v")
        nc.vector.tensor_tensor(v[:], a_cur[:], b_cur[:], OP.mult)
        nc.tensor.matmul(fin[:], ones20[:], v[:], start=True, stop=True)
        lnz = crfp.tile([1, BL], F32, tag="lnz")
        nc.scalar.activation(lnz[:], fin[:], AF.Ln)
        sc = fips.tile([1, BL], F32, tag="sc")
        nc.tensor.matmul(sc[:], ones20[:], unacc[:], start=True, stop=False)
        nc.tensor.matmul(sc[:], ones20[:], binacc[:], start=False, stop=True)
        res = crfp.tile([1, BL], F32, tag="res")
        nc.vector.scalar_tensor_tensor(res[:], lnz[:], -1.0, sc[:], OP.mult, OP.add)
        res2 = crfp.tile([1, BL], F32, tag="res2")
        nc.vector.tensor_scalar_add(res2[:], res[:], -float(T) * SIGMA)
        nc.sync.dma_start(io["out_ll"][:], res2[:])


# ---------------------------------------------------------------- host packing

def _bf(x):
    return np.ascontiguousarray(x, dtype=BFNP)


def _f32(x):
    return np.ascontiguousarray(x, dtype=np.float32)


def pack_shared(w, T):
    """Shared (replicated) weight arrays -> dict of np arrays."""
    out = {}
    convp = np.zeros((D, NPAIR * 128), np.float32)
    ws = [w["w1"], w["w2"], w["w3"], w["w4"]]  # [K, D, C]
    # channel block ch0 of conv j starts at j*C in the concat
    for p, (mb, off) in enumerate(PAIRS):
        lo, hi = mb * 128, (mb + 1) * 128
        for j, wj in enumerate(ws):
            Kj = wj.shape[0]
            pad_l = (Kj - 1) // 2
            c0, c1 = j * C, (j + 1) * C
            s, e = max(lo, c0), min(hi, c1)
            if s >= e:
                continue
            kk = off + pad_l  # tap index within this conv
            if 0 <= kk < Kj:
                convp[:, p * 128 + (s - lo): p * 128 + (e - lo)] = wj[kk][:, s - c0:e - c0]
    if FP8_CONV:
        out["convp"] = np.ascontiguousarray(convp * WSC, dtype=FP8NP)
        out["g1"] = _f32(w["ln1_g"].reshape(KD, 128).T * XSC)
        out["b1"] = _f32(w["ln1_b"].reshape(KD, 128).T * XSC)
    else:
        out["convp"] = _bf(convp)
        out["g1"] = _f32(w["ln1_g"].reshape(KD, 128).T)
        out["b1"] = _f32(w["ln1_b"].reshape(KD, 128).T)
    ln1b = np.asarray(w["ln1_b"], np.float64)
    bconv = np.concatenate([
        np.broadcast_to(w[f"b{j + 1}"], (C,)).astype(np.float64)
        + np.einsum("kdc,d->c", np.asarray(w[f"w{j + 1}"], np.float64), ln1b)
        for j in range(4)])
    out["bconv"] = _f32(bconv.reshape(6, 128).T)
    out["g2"] = _f32(w["ln2_g"].reshape(KC, 128).T * (XSC2 if FP8_XW else 1.0))
    out["b2"] = _f32(w["ln2_b"].reshape(KC, 128).T)
    # g-gate columns (last H after PERM) are scaled by 2 so the device uses
    # one sigmoid over all gates: tanh(x) = 2*sigmoid(2x) - 1
    gsc = np.ones(G4, np.float32)
    gsc[3 * H:] = 2.0
    wx_cat = np.concatenate(
        [w["wx_f"][:, PERM] * gsc, w["wx_b"][:, PERM] * gsc], axis=1)
    if FP8_XW:
        out["wx"] = np.ascontiguousarray(np.asarray(wx_cat) * WSC2, dtype=FP8NP)
    else:
        out["wx"] = _bf(wx_cat)
    wh_cat = np.concatenate(
        [w["wh_f"][:, PERM] * gsc, w["wh_b"][:, PERM] * gsc], axis=1)
    # fp8 Wh unscaled: values ~N(0,.02) land partly subnormal, which numpy
    # simulation shows costs < 5e-5 end-to-end rel err
    out["wh"] = np.ascontiguousarray(np.asarray(wh_cat), dtype=FP8NP) \
        if FP8_H else _bf(wh_cat)
    ln2b = np.asarray(w["ln2_b"], np.float64)
    sh_f = ln2b @ np.asarray(w["wx_f"], np.float64)[:, PERM]
    sh_b = ln2b @ np.asarray(w["wx_b"], np.float64)[:, PERM]
    bz = np.stack([(w["bf"][PERM] + sh_f) * gsc,
                   (w["bb"][PERM] + sh_b) * gsc]).astype(np.float32).reshape(2, MG, 128)
    out["bz"] = _f32(np.moveaxis(bz, 2, 0))  # [128, 2, MG]
    if FP8_H:
        # wd scaled x64 into fp8 range; em PSUM is then 64x -> the Exp ACT
        # de-scales via scale=1/WDSC, and the unary-score STT compensates via
        # bd*64 and onehot/64 (dump = (em' + 64 bd) * (oh/64))
        out["wd"] = np.ascontiguousarray(np.asarray(w["wd"]) * WDSC, dtype=FP8NP)
        out["bd"] = _f32(w["bd"].reshape(LBL, 1) * WDSC)
    else:
        out["wd"] = _bf(w["wd"])
        out["bd"] = _f32(w["bd"].reshape(LBL, 1))
    out["bdm"] = _f32(w["bd"].reshape(LBL, 1) - SIGMA)
    out["trans"] = _f32(w["trans"])
    out["transT"] = _f32(np.asarray(w["trans"]).T)
    return out


def pack_core(hid_a, hid_b, targets, c0, T):
    """Per-core data arrays for batch slice [c0, c0+BL)."""
    out = {}
    ha = np.asarray(hid_a[c0:c0 + BL])  # [BL, T, D_BERT]
    hb = np.asarray(hid_b[c0:c0 + BL])
    hidT = np.empty((D, BL, T), BFNP)
    hidT[:D_BERT] = ha.transpose(2, 0, 1)
    hidT[D_BERT:] = hb.transpose(2, 0, 1)
    out["hidT"] = hidT
    tg = np.asarray(targets[c0:c0 + BL])  # [BL, T] int32
    oh = np.zeros((BL, T, LBL), np.float32)
    np.put_along_axis(oh, tg[..., None], 1.0, axis=2)
    out["ohT"] = _f32(oh.transpose(2, 0, 1) * (1.0 / WDSC if FP8_H else 1.0))
    ohp = np.zeros((BL, T, LBL), BFNP)
    ohn = np.zeros((BL, T, LBL), BFNP)
    ohp[:, :T - 1] = oh[:, :T - 1]
    ohn[:, :T - 1] = oh[:, 1:]
    out["ohp"] = ohp
    out["ohn"] = ohn
    return out


# ---------------------------------------------------------------- numpy oracle

def numpy_reference(inputs, attention_mask, targets, hid_a, hid_b, ln1_g, ln1_b,
                    w1, b1, w2, b2, w3, b3, w4, b4, ln2_g, ln2_b,
                    wx_f, wh_f, bf, wx_b, wh_b, bb, wd, bd, trans):
    """Pure-numpy double-precision port of reference.py (general fallback)."""
    def ln(x, g, b):
        m = x.mean(-1, keepdims=True)
        v = ((x - m) ** 2).mean(-1, keepdims=True)
        return (x - m) / np.sqrt(v + EPS) * g + b

    def conv1d_relu(x, w, b):
        K = w.shape[0]
        pad_l = (K - 1) // 2
        Bn, Tn, Din = x.shape
        xp = np.zeros((Bn, Tn + K - 1, Din), x.dtype)
        xp[:, pad_l:pad_l + Tn] = x
        y = np.zeros((Bn, Tn, w.shape[2]), x.dtype)
        for k in range(K):
            y += xp[:, k:k + Tn] @ w[k]
        return np.maximum(y + b, 0.0)

    def sig(x):
        return 1.0 / (1.0 + np.exp(-x))

    def lstm(x, mask, Wx, Wh, bias, reverse):
        Bn, Tn, _ = x.shape
        Hn = Wh.shape[0]
        h = np.zeros((Bn, Hn), x.dtype)
        c = np.zeros((Bn, Hn), x.dtype)
        op = np.zeros((Bn, Hn), x.dtype)
        ys = np.zeros((Bn, Tn, Hn), x.dtype)
        order = range(Tn - 1, -1, -1) if reverse else range(Tn)
        for t in order:
            z = x[:, t] @ Wx + h @ Wh + bias
            i, f, g, o = np.split(z, 4, axis=-1)
            i, f, o = sig(i), sig(f), sig(o)
            cn = f * c + i * np.tanh(g)
            hn = o * np.tanh(cn)
            m = mask[:, t][:, None]
            h = np.where(m, hn, h)
            c = np.where(m, cn, c)
            op = np.where(m, hn, op)
            ys[:, t] = op
        return ys

    x = np.concatenate([np.asarray(hid_a, np.float64), np.asarray(hid_b, np.float64)], axis=-1)
    x = ln(x, np.asarray(ln1_g, np.float64), np.asarray(ln1_b, np.float64))
    conv = np.concatenate([
        conv1d_relu(x, np.asarray(w1, np.float64), b1),
        conv1d_relu(x, np.asarray(w2, np.float64), b2),
        conv1d_relu(x, np.asarray(w3, np.float64), b3),
        conv1d_relu(x, np.asarray(w4, np.float64), b4)], axis=-1)
    conv = ln(conv, np.asarray(ln2_g, np.float64), np.asarray(ln2_b, np.float64))
    mask = np.asarray(attention_mask) != 0
    hf = lstm(conv, mask, np.asarray(wx_f, np.float64), np.asarray(wh_f, np.float64),
              np.asarray(bf, np.float64), False)
    hbk = lstm(conv, mask, np.asarray(wx_b, np.float64), np.asarray(wh_b, np.float64),
               np.asarray(bb, np.float64), True)
    h = np.concatenate([hf, hbk], axis=-1)
    logits = h @ np.asarray(wd, np.float64) + np.asarray(bd, np.float64)
    seq_len = (np.asarray(inputs) != 0).astype(np.int64).sum(1)
    Bn, Tn, L = logits.shape
    tg = np.asarray(targets)
    valid = np.arange(Tn)[None, :] < seq_len[:, None]
    unary = np.take_along_axis(logits, tg[..., None], axis=2)[..., 0]
    unary_score = np.where(valid, unary, 0.0).sum(1)
    pair = np.asarray(trans, np.float64)[tg[:, :-1], tg[:, 1:]]
    binary_score = np.where(valid[:, 1:], pair, 0.0).sum(1)
    alpha = logits[:, 0]
    tr = np.asarray(trans, np.float64)
    for t in range(1, Tn):
        nxt = alpha[:, :, None] + tr[None, :, :]
        mx = nxt.max(1)
        nxt = np.log(np.exp(nxt - mx[:, None, :]).sum(1)) + mx + logits[:, t]
        alpha = np.where(valid[:, t][:, None], nxt, alpha)
    mx = alpha.max(1)
    log_norm = np.log(np.exp(alpha - mx[:, None]).sum(1)) + mx
    return (unary_score + binary_score - log_norm).astype(np.float32)


# ---------------------------------------------------------------- program build

_CACHE = {}


def build_program(T=T_FULL, TCH=64):
    key = (T, TCH)
    if key in _CACHE:
        return _CACHE[key]
    nc = bacc.Bacc("TRN2", target_bir_lowering=False, debug=False,
                   enable_asserts=False, num_devices=NCORE)
    io = {}

    def din(name, shape, dt):
        io[name] = nc.dram_tensor(name, shape, dt, kind="ExternalInput").ap()

    din("hidT", [D, BL, T], BF16)
    din("convp", [D, NPAIR * 128], FP8 if FP8_CONV else BF16)
    din("bconv", [128, 6], F32)
    din("g1", [128, KD], F32)
    din("b1", [128, KD], F32)
    din("g2", [128, KC], F32)
    din("b2", [128, KC], F32)
    din("wx", [C4, 2 * G4], FP8 if FP8_XW else BF16)
    din("wh", [H, 2 * G4], FP8 if FP8_H else BF16)
    din("bz", [128, 2, MG], F32)
    din("wd", [2 * H, LBL], FP8 if FP8_H else BF16)
    din("bd", [LBL, 1], F32)
    din("bdm", [LBL, 1], F32)
    din("trans", [LBL, LBL], F32)
    din("transT", [LBL, LBL], F32)
    din("ohT", [LBL, BL, T], F32)
    din("ohp", [BL, T, LBL], BF16)
    din("ohn", [BL, T, LBL], BF16)
    io["out_ll"] = nc.dram_tensor("out_ll", [1, BL], F32, kind="ExternalOutput").ap()

    with tile.TileContext(nc) as tc:
        _emit(tc, io, T, TCH)
    nc.compile()
    _CACHE[key] = nc
    return nc


# ---------------------------------------------------------------- entry point

TRACE = False          # set True (e.g. from test.py) to capture an NTFF profile
LAST_RESULTS = None    # BassKernelResults of the most recent run


def kernel(**inputs):
    global LAST_RESULTS
    inputs = {k: np.asarray(v) for k, v in inputs.items()}
    if (inputs["inputs"] == 0).any() or (inputs["attention_mask"] == 0).any():
        # out-of-distribution (masked) input: exact host fallback
        return numpy_reference(**inputs)

    from concourse.bass_utils import run_bass_kernel_spmd

    T = inputs["inputs"].shape[1]
    nc = build_program(T=T)
    shared = pack_shared(inputs, T)
    in_maps = []
    for core in range(NCORE):
        m = dict(shared)
        m.update(pack_core(inputs["hid_a"], inputs["hid_b"], inputs["targets"],
                           core * BL, T))
        in_maps.append(m)
    res = run_bass_kernel_spmd(nc, in_maps, core_ids=list(range(NCORE)), trace=TRACE)
    LAST_RESULTS = res
    out = np.concatenate([res.results[c]["out_ll"][0] for c in range(NCORE)])
    return out.astype(np.float32)


if __name__ == "__main__":
    print("kernel module ok")

